# revision 5
# baseline (speedup 1.0000x reference)
"""Bass/Trainium2 kernel for nn_LowRankLoss.

Reference computation:
  m      = mean(feat, axis=1)                      # [n, h, w], channel mean
  normed = m / ||m||_F (per sample)
  rank   = #(singular values of normed > 0)        # [n]
  loss   = sum(max(0, -(rank1 - rank2))) / n

The memory-bound part (target_regime=memory) is the channel-mean reduction
over two [128, 256, 32, 64] f32 tensors (512 MiB total). That runs on 8
NeuronCores, data-parallel over the batch dim (16 samples/core). The device
returns per-sample channel sums [n, 2048]; the tiny per-sample SVDs
(128 matrices of 32x64) and the scalar loss are finished on host.

Device design per core (per input tensor, viewed [NS=16, 2, 128, F=2048]):
  - One fully contiguous 2 MiB DMA per sample -> SBUF [128, 4096]
    (channel half cb in free cols [cb*F, (cb+1)*F)). Contiguity matters:
    strided gathers measured at half HBM bandwidth (178 vs 342 GB/s).
    SWDGE (gpsimd) issues all input DMAs - it spreads across all 16 SDMA
    engines; the HWDGE rings only reach 8 of them.
  - VectorE folds the two channel halves (t[:, :F] + t[:, F:]) and rounds
    to fp32r for the PE (fp32r moving streams 1 cycle/row vs 4 for fp32).
  - TensorE reduces the remaining 128 channels (partition dim) per sample:
    stationary S_m [128, 8] is all-ones in column m = s%8 and zero
    elsewhere, so sample s lands in PSUM row m while other rows accumulate
    +0. Eight samples share one PSUM tile [8, F] (one accumulation group
    per 512-col bank chunk).
  - acc [8, F] -> SBUF via VectorE -> 64 KiB DMA out per group.
fp32r truncates the data mantissa (~1e-4 rel err), far below what could
flip a singular-value-positivity count (min sigma ~2e-2 here).
"""

import numpy as np

N_CORES = 8
NS = 16           # samples per core
C = 256           # channels
H, W = 32, 64
F = H * W         # 2048 spatial
CB = 2            # channel halves
P = 128           # partitions
SG = 8            # samples per PSUM group
NB = 4            # matmuls per sample (N=512 PSUM bank limit)
BN = F // NB      # 512

_CACHE = {}


def _build_nc():
    import concourse.bacc as bacc
    import concourse.mybir as mybir
    import concourse.tile as tile

    nc = bacc.Bacc(None, target_bir_lowering=False)
    f32 = mybir.dt.float32
    f32r = mybir.dt.float32r

    x_raw = nc.dram_tensor("x_raw", [NS, CB, P, F], f32, kind="ExternalInput")
    x_rect = nc.dram_tensor("x_rect", [NS, CB, P, F], f32, kind="ExternalInput")
    out_raw = nc.dram_tensor("out_raw", [NS, F], f32, kind="ExternalOutput")
    out_rect = nc.dram_tensor("out_rect", [NS, F], f32, kind="ExternalOutput")

    with tile.TileContext(nc) as tc:
        with (
            tc.tile_pool(name="io", bufs=8) as pool,
            tc.tile_pool(name="red", bufs=3) as redp,
            tc.tile_pool(name="small", bufs=2) as small,
            tc.tile_pool(name="psum", bufs=2, space="PSUM") as psum,
        ):
            # C[k, 8m + j] = 1 if j == m else 0; lhsT for sample s is the
            # [128, 8] slice C[:, 8m:8m+8] with m = s % 8.
            s_np = np.zeros((P, SG * SG), np.float32)
            for m in range(SG):
                s_np[:, SG * m + m] = 1.0
            s_dram = nc.inline_tensor(s_np, name="s_const")
            s_stage = small.tile([P, SG * SG], f32, tag="stat_stage")
            nc.sync.dma_start(s_stage[:], s_dram[:])
            S = small.tile([P, SG * SG], f32r, tag="stat")
            nc.vector.tensor_copy(S[:], s_stage[:])

            for xt, ot in ((x_raw, out_raw), (x_rect, out_rect)):
                for g in range(NS // SG):
                    acc = psum.tile([SG, F], f32, tag="acc")
                    for m in range(SG):
                        s = g * SG + m
                        # two contiguous 1 MiB transfers per sample
                        t0 = pool.tile([P, F], f32, tag="in0")
                        t1 = pool.tile([P, F], f32, tag="in1")
                        # first tile pair rides the HWDGE rings (shorter
                        # first-byte) while the Q7 SWDGE path spins up
                        if s == 0 and xt is x_raw:
                            nc.sync.dma_start(t0[:], xt[s, 0])
                            nc.scalar.dma_start(t1[:], xt[s, 1])
                        else:
                            nc.gpsimd.dma_start(t0[:], xt[s, 0])
                            nc.gpsimd.dma_start(t1[:], xt[s, 1])
                        # fold channel halves + round to fp32r for the PE
                        tr = redp.tile([P, F], f32r, tag="red")
                        nc.vector.tensor_add(tr[:], t0[:], t1[:])
                        for j in range(NB):
                            nc.tensor.matmul(
                                acc[:, j * BN : (j + 1) * BN],
                                S[:, SG * m : SG * m + SG],
                                tr[:, j * BN : (j + 1) * BN],
                                start=(m == 0),
                                stop=(m == SG - 1),
                            )
                    osb = small.tile([SG, F], f32, tag="osb")
                    nc.vector.tensor_copy(osb[:], acc[:])
                    nc.sync.dma_start(ot[g * SG : (g + 1) * SG], osb[:])

    nc.compile()
    return nc


def _device_channel_sums(raw, rect, trace=False):
    """Run the bass kernel on 8 cores; return (sums_raw, sums_rect) [128, 2048]
    and the BassKernelResults."""
    from concourse.bass_utils import run_bass_kernel_spmd

    if "nc" not in _CACHE:
        _CACHE["nc"] = _build_nc()
    nc = _CACHE["nc"]

    raw5 = raw.reshape(N_CORES, NS, CB, P, F)
    rect5 = rect.reshape(N_CORES, NS, CB, P, F)
    in_maps = [{"x_raw": raw5[i], "x_rect": rect5[i]} for i in range(N_CORES)]
    res = run_bass_kernel_spmd(nc, in_maps, list(range(N_CORES)), trace=trace)

    sums_raw = np.concatenate([res.results[i]["out_raw"] for i in range(N_CORES)])
    sums_rect = np.concatenate([res.results[i]["out_rect"] for i in range(N_CORES)])
    return sums_raw, sums_rect, res


def _rank_from_sums(sums):
    # channel mean (exact: /256 is a power of two), normalize, svd, count
    m = (sums / np.float32(C)).astype(np.float32)
    nrm = np.linalg.norm(m, axis=1, keepdims=True)
    normed = (m / nrm).reshape(-1, H, W)
    s = np.linalg.svd(normed.astype(np.float32), compute_uv=False)
    return (s > 0.0).sum(axis=1).astype(np.float32)


def kernel(raw_feat, rectified_feat, trace=False):
    raw = np.ascontiguousarray(np.asarray(raw_feat, dtype=np.float32))
    rect = np.ascontiguousarray(np.asarray(rectified_feat, dtype=np.float32))

    sums_raw, sums_rect, res = _device_channel_sums(raw, rect, trace=trace)
    _CACHE["last_results"] = res
    _CACHE["last_sums"] = (sums_raw, sums_rect)

    rank1 = _rank_from_sums(sums_raw)
    rank2 = _rank_from_sums(sums_rect)
    loss = np.maximum(np.float32(0.0), -(rank1 - rank2))
    loss = loss.sum(dtype=np.float32) / np.float32(raw.shape[0])
    return np.asarray(loss, dtype=np.float32)



# revision 7
# speedup vs baseline: 1.0460x; 1.0460x over previous
"""Bass/Trainium2 kernel for nn_LowRankLoss.

Reference computation:
  m      = mean(feat, axis=1)                      # [n, h, w], channel mean
  normed = m / ||m||_F (per sample)
  rank   = #(singular values of normed > 0)        # [n]
  loss   = sum(max(0, -(rank1 - rank2))) / n

The memory-bound part (target_regime=memory) is the channel-mean reduction
over two [128, 256, 32, 64] f32 tensors (512 MiB total). That runs on 8
NeuronCores, data-parallel over the batch dim (16 samples/core). The device
returns per-sample channel sums [n, 2048]; the tiny per-sample SVDs
(128 matrices of 32x64) and the scalar loss are finished on host.

Device design per core (per input tensor, viewed [NS=16, 2, 128, F=2048]):
  - Two contiguous 1 MiB DMAs per sample -> SBUF [128, 2048] x2. 8 KiB
    per-partition descriptors: measured optimal (16 KiB descriptors
    trigger a ~20% slowdown on SDMA engine 15 - the SWDGE descriptor-ring
    port pathology - and the statically balanced work then drags the whole
    stream from 164 us to 199 us). SWDGE (gpsimd) issues the steady-state
    input DMAs across all 16 SDMA engines; sample 0 rides the HWDGE rings
    (sync/scalar, shorter first-byte) while the Q7 SWDGE path spins up,
    and is issued BEFORE the one-hot stationary load so input bytes flow
    at the earliest point.
  - VectorE folds the two channel halves (t0 + t1) and rounds to fp32r
    for the PE (fp32r moving streams 1 cycle/row vs 4 for fp32).
  - TensorE reduces the remaining 128 channels (partition dim) per sample:
    stationary S_m [128, 8] is all-ones in column m = s%8 and zero
    elsewhere, so sample s lands in PSUM row m while other rows accumulate
    +0. Eight samples share one PSUM accumulation group per 512-col bank.
  - acc -> SBUF via VectorE -> DMA out per group.
  - Tail: everything after the last input byte is serial latency, so the
    globally last sample (x_rect s15) is pipelined in 512-column chunks:
    chunked strided DMAs -> chunked fold -> chunked stop-matmul into
    per-bank PSUM tiles -> chunked PSUM->SBUF copies -> chunked output
    DMAs (final chunk on the scalar HWDGE queue so it does not FIFO
    behind the earlier output chunks). This cuts the post-stream serial
    chain from ~9.6 us (2.3 ADD + 1.9 MM + 2.3 COPY + 1.2 out-DMA) to
    ~3.5 us. The remaining ~8.5 us after the last output DMA is the
    framework's fixed epilogue (DMA-lane drains, two all-engine barriers,
    per-engine semaphore-file resets) and does not depend on the kernel
    body.
fp32r truncates the data mantissa (~1e-4 rel err), far below what could
flip a singular-value-positivity count (min sigma ~2e-2 here).
"""

import numpy as np

N_CORES = 8
NS = 16           # samples per core
C = 256           # channels
H, W = 32, 64
F = H * W         # 2048 spatial
CB = 2            # channel halves
P = 128           # partitions
SG = 8            # samples per PSUM group
NB = 4            # matmuls per sample (N=512 PSUM bank limit)
BN = F // NB      # 512

_CACHE = {}


def _build_nc():
    import concourse.bacc as bacc
    import concourse.mybir as mybir
    import concourse.tile as tile

    nc = bacc.Bacc(None, target_bir_lowering=False)
    f32 = mybir.dt.float32
    f32r = mybir.dt.float32r

    x_raw = nc.dram_tensor("x_raw", [NS, CB, P, F], f32, kind="ExternalInput")
    x_rect = nc.dram_tensor("x_rect", [NS, CB, P, F], f32, kind="ExternalInput")
    out_raw = nc.dram_tensor("out_raw", [NS, F], f32, kind="ExternalOutput")
    out_rect = nc.dram_tensor("out_rect", [NS, F], f32, kind="ExternalOutput")

    with tile.TileContext(nc) as tc:
        with (
            tc.tile_pool(name="io", bufs=8) as pool,
            tc.tile_pool(name="red", bufs=3) as redp,
            tc.tile_pool(name="small", bufs=2) as small,
            tc.tile_pool(name="psum", bufs=1, space="PSUM") as psum,
        ):
            # Sample 0 rides the HWDGE rings before anything else so input
            # bytes start flowing at the earliest possible point.
            t0_first = pool.tile([P, F], f32, tag="in0")
            t1_first = pool.tile([P, F], f32, tag="in1")
            nc.sync.dma_start(t0_first[:], x_raw[0, 0])
            nc.scalar.dma_start(t1_first[:], x_raw[0, 1])

            # C[k, 8m + j] = 1 if j == m else 0; lhsT for sample s is the
            # [128, 8] slice C[:, 8m:8m+8] with m = s % 8.
            s_np = np.zeros((P, SG * SG), np.float32)
            for m in range(SG):
                s_np[:, SG * m + m] = 1.0
            s_dram = nc.inline_tensor(s_np, name="s_const")
            s_stage = small.tile([P, SG * SG], f32, tag="stat_stage")
            nc.sync.dma_start(s_stage[:], s_dram[:])
            S = small.tile([P, SG * SG], f32r, tag="stat")
            nc.vector.tensor_copy(S[:], s_stage[:])

            for xt, ot in ((x_raw, out_raw), (x_rect, out_rect)):
                for g in range(NS // SG):
                    last_group = xt is x_rect and g == NS // SG - 1
                    if not last_group:
                        acc = psum.tile([SG, F], f32, tag="acc", bufs=1)
                    else:
                        # per-bank PSUM tiles so each 512-col chunk's copy
                        # only waits on its own accumulation chain
                        accc = [
                            psum.tile([SG, BN], f32, tag=f"accc{j}", bufs=1,
                                      name=f"accc{j}")
                            for j in range(NB)
                        ]
                    for m in range(SG):
                        s = g * SG + m
                        if last_group and m == SG - 1:
                            break  # chunked tail below
                        if s == 0 and xt is x_raw:
                            t0, t1 = t0_first, t1_first
                        else:
                            t0 = pool.tile([P, F], f32, tag="in0")
                            t1 = pool.tile([P, F], f32, tag="in1")
                            nc.gpsimd.dma_start(t0[:], xt[s, 0])
                            nc.gpsimd.dma_start(t1[:], xt[s, 1])
                        # fold channel halves + round to fp32r for the PE
                        tr = redp.tile([P, F], f32r, tag="red")
                        nc.vector.tensor_add(tr[:], t0[:], t1[:])
                        for j in range(NB):
                            dst = acc[:, j * BN : (j + 1) * BN] if not last_group else accc[j][:]
                            nc.tensor.matmul(
                                dst,
                                S[:, SG * m : SG * m + SG],
                                tr[:, j * BN : (j + 1) * BN],
                                start=(m == 0),
                                stop=(m == SG - 1) and not last_group,
                            )
                    if not last_group:
                        osb = small.tile([SG, F], f32, tag="osb")
                        nc.vector.tensor_copy(osb[:], acc[:])
                        nc.sync.dma_start(ot[g * SG : (g + 1) * SG], osb[:])
                    else:
                        # ---- chunked tail for the globally last sample ----
                        # All DMAs+folds+stop-matmuls first so DVE's in-order
                        # queue is ADD0..ADD3,COPY3 and the last chunk's fold
                        # runs the moment its data lands; chunks 0-2 copy on
                        # ACT so they never block DVE.
                        s = g * SG + SG - 1
                        m = SG - 1
                        rows = ot[g * SG : (g + 1) * SG]
                        for j in range(NB):
                            c0, c1 = j * BN, (j + 1) * BN
                            ta = pool.tile([P, BN], f32, tag="chka", bufs=4,
                                           name=f"ta{j}")
                            tb = pool.tile([P, BN], f32, tag="chkb", bufs=4,
                                           name=f"tb{j}")
                            nc.gpsimd.dma_start(ta[:], xt[s, 0, :, c0:c1])
                            nc.gpsimd.dma_start(tb[:], xt[s, 1, :, c0:c1])
                            trc = redp.tile([P, BN], f32r, tag="redc", bufs=4,
                                            name=f"trc{j}")
                            nc.vector.tensor_add(trc[:], ta[:], tb[:])
                            nc.tensor.matmul(
                                accc[j][:],
                                S[:, SG * m : SG * m + SG],
                                trc[:],
                                start=False,
                                stop=True,
                            )
                        for j in range(NB):
                            c0, c1 = j * BN, (j + 1) * BN
                            osbc = small.tile([SG, BN], f32, tag="osbc", bufs=4,
                                              name=f"osbc{j}")
                            if j == NB - 1:
                                # critical path: DVE copy + the uncontended
                                # scalar HWDGE queue for the final output
                                nc.vector.tensor_copy(osbc[:], accc[j][:])
                                nc.scalar.dma_start(rows[:, c0:c1], osbc[:])
                            else:
                                nc.scalar.copy(osbc[:], accc[j][:])
                                nc.sync.dma_start(rows[:, c0:c1], osbc[:])

    nc.compile()
    return nc


def _device_channel_sums(raw, rect, trace=False):
    """Run the bass kernel on 8 cores; return (sums_raw, sums_rect) [128, 2048]
    and the BassKernelResults."""
    from concourse.bass_utils import run_bass_kernel_spmd

    if "nc" not in _CACHE:
        _CACHE["nc"] = _build_nc()
    nc = _CACHE["nc"]

    raw5 = raw.reshape(N_CORES, NS, CB, P, F)
    rect5 = rect.reshape(N_CORES, NS, CB, P, F)
    in_maps = [{"x_raw": raw5[i], "x_rect": rect5[i]} for i in range(N_CORES)]
    res = run_bass_kernel_spmd(nc, in_maps, list(range(N_CORES)), trace=trace)

    sums_raw = np.concatenate([res.results[i]["out_raw"] for i in range(N_CORES)])
    sums_rect = np.concatenate([res.results[i]["out_rect"] for i in range(N_CORES)])
    return sums_raw, sums_rect, res


def _rank_from_sums(sums):
    # channel mean (exact: /256 is a power of two), normalize, svd, count
    m = (sums / np.float32(C)).astype(np.float32)
    nrm = np.linalg.norm(m, axis=1, keepdims=True)
    normed = (m / nrm).reshape(-1, H, W)
    s = np.linalg.svd(normed.astype(np.float32), compute_uv=False)
    return (s > 0.0).sum(axis=1).astype(np.float32)


def kernel(raw_feat, rectified_feat, trace=False):
    raw = np.ascontiguousarray(np.asarray(raw_feat, dtype=np.float32))
    rect = np.ascontiguousarray(np.asarray(rectified_feat, dtype=np.float32))

    sums_raw, sums_rect, res = _device_channel_sums(raw, rect, trace=trace)
    _CACHE["last_results"] = res
    _CACHE["last_sums"] = (sums_raw, sums_rect)

    rank1 = _rank_from_sums(sums_raw)
    rank2 = _rank_from_sums(sums_rect)
    loss = np.maximum(np.float32(0.0), -(rank1 - rank2))
    loss = loss.sum(dtype=np.float32) / np.float32(raw.shape[0])
    return np.asarray(loss, dtype=np.float32)


# revision 9
# speedup vs baseline: 1.0603x; 1.0137x over previous
"""Bass/Trainium2 kernel for nn_LowRankLoss.

Reference computation:
  m      = mean(feat, axis=1)                      # [n, h, w], channel mean
  normed = m / ||m||_F (per sample)
  rank   = #(singular values of normed > 0)        # [n]
  loss   = sum(max(0, -(rank1 - rank2))) / n

The memory-bound part (target_regime=memory) is the channel-mean reduction
over two [128, 256, 32, 64] f32 tensors (512 MiB total). That runs on 8
NeuronCores, data-parallel over the batch dim (16 samples/core). The device
returns per-sample channel sums [n, 2048]; the tiny per-sample SVDs
(128 matrices of 32x64) and the scalar loss are finished on host.

Device design per core (per input tensor, viewed [NS=16, 2, 128, F=2048]):
  - Two contiguous 1 MiB DMAs per sample -> SBUF [128, 2048] x2. 8 KiB
    per-partition descriptors: measured optimal (16 KiB descriptors
    trigger a ~20% slowdown on SDMA engine 15 - the SWDGE descriptor-ring
    port pathology - and the statically balanced work then drags the
    whole stream). SWDGE (gpsimd) issues the steady-state input DMAs
    across all 16 SDMA engines; sample 0 rides the HWDGE rings
    (sync/scalar, shorter first-byte) while the Q7 SWDGE path spins up,
    and is issued BEFORE the one-hot stationary load so input bytes flow
    at the earliest point.
  - TensorE reduces all 256 channels per sample directly from the raw
    input tiles: the two channel-half tiles are bitcast f32->f32r (same
    bits; the PE truncates the mantissa while streaming 1 cycle/row vs 4
    for fp32) and each contributes 4 bank-chunk matmuls accumulated in
    PSUM. Stationary S_m [128, 8] is all-ones in column m = s%8 and zero
    elsewhere, so sample s lands in PSUM row m while other rows
    accumulate +0. No VectorE fold at all: 16-matmul accumulation chains
    (8 samples x 2 halves) per 512-col PSUM bank.
  - acc -> SBUF copy -> DMA out per group.
  - Tail: everything after the last input byte is serial latency. The
    last group uses per-bank PSUM tiles; after the final 1 MiB tile
    lands, only 4 chunk matmuls + per-bank copies + chunked output DMAs
    remain. Chunks 0-2 copy on ACT (scalar) so the final chunk's DVE
    copy is never queued, and the final output chunk goes out on its own
    HWDGE queue. The remaining ~8.5 us after the last output DMA is the
    fixed framework epilogue (DMA-lane drains, all-engine barriers, a
    constant ~4.2 us quiesce wait, semaphore-file resets) and does not
    depend on the kernel body.
f32r truncates the data mantissa (~1e-4 rel err on the channel sums),
far below what could flip a singular-value-positivity count (min sigma
~2e-2 here).
"""

import numpy as np

N_CORES = 8
NS = 16           # samples per core
C = 256           # channels
H, W = 32, 64
F = H * W         # 2048 spatial
CB = 2            # channel halves
P = 128           # partitions
SG = 8            # samples per PSUM group
NB = 4            # matmuls per half-sample (N=512 PSUM bank limit)
BN = F // NB      # 512

_CACHE = {}


def _build_nc():
    import concourse.bacc as bacc
    import concourse.mybir as mybir
    import concourse.tile as tile

    nc = bacc.Bacc(None, target_bir_lowering=False)
    f32 = mybir.dt.float32
    f32r = mybir.dt.float32r

    x_raw = nc.dram_tensor("x_raw", [NS, CB, P, F], f32, kind="ExternalInput")
    x_rect = nc.dram_tensor("x_rect", [NS, CB, P, F], f32, kind="ExternalInput")
    out_raw = nc.dram_tensor("out_raw", [NS, F], f32, kind="ExternalOutput")
    out_rect = nc.dram_tensor("out_rect", [NS, F], f32, kind="ExternalOutput")

    with tile.TileContext(nc) as tc:
        with (
            tc.tile_pool(name="io", bufs=9) as pool,
            tc.tile_pool(name="small", bufs=2) as small,
            tc.tile_pool(name="psum", bufs=1, space="PSUM") as psum,
        ):
            # Sample 0 rides the HWDGE rings before anything else so input
            # bytes start flowing at the earliest possible point.
            t0_first = pool.tile([P, F], f32r, tag="in0")
            t1_first = pool.tile([P, F], f32r, tag="in1")
            nc.sync.dma_start(t0_first[:], x_raw[0, 0].bitcast(f32r))
            nc.scalar.dma_start(t1_first[:], x_raw[0, 1].bitcast(f32r))

            # C[k, 8m + j] = 1 if j == m else 0; lhsT for sample s is the
            # [128, 8] slice C[:, 8m:8m+8] with m = s % 8.
            s_np = np.zeros((P, SG * SG), np.float32)
            for m in range(SG):
                s_np[:, SG * m + m] = 1.0
            s_dram = nc.inline_tensor(s_np, name="s_const")
            s_stage = small.tile([P, SG * SG], f32, tag="stat_stage")
            nc.sync.dma_start(s_stage[:], s_dram[:])
            S = small.tile([P, SG * SG], f32r, tag="stat")
            nc.vector.tensor_copy(S[:], s_stage[:])

            for xt, ot in ((x_raw, out_raw), (x_rect, out_rect)):
                for g in range(NS // SG):
                    last_group = xt is x_rect and g == NS // SG - 1
                    if not last_group:
                        acc = psum.tile([SG, F], f32, tag="acc", bufs=1)
                        accb = [acc[:, j * BN : (j + 1) * BN] for j in range(NB)]
                    else:
                        # per-bank PSUM tiles so each 512-col chunk's copy
                        # only waits on its own accumulation chain
                        accb = [
                            psum.tile([SG, BN], f32, tag=f"accc{j}", bufs=1,
                                      name=f"accc{j}")
                            for j in range(NB)
                        ]
                    for m in range(SG):
                        s = g * SG + m
                        if s == 0 and xt is x_raw:
                            t0, t1 = t0_first, t1_first
                        else:
                            t0 = pool.tile([P, F], f32r, tag="in0")
                            t1 = pool.tile([P, F], f32r, tag="in1")
                            nc.gpsimd.dma_start(t0[:], xt[s, 0].bitcast(f32r))
                            nc.gpsimd.dma_start(t1[:], xt[s, 1].bitcast(f32r))
                        for h, t in ((0, t0), (1, t1)):
                            tm = t
                            for j in range(NB):
                                nc.tensor.matmul(
                                    accb[j],
                                    S[:, SG * m : SG * m + SG],
                                    tm[:, j * BN : (j + 1) * BN],
                                    start=(m == 0 and h == 0),
                                    stop=(m == SG - 1 and h == 1),
                                )
                    if not last_group:
                        osb = small.tile([SG, F], f32, tag="osb")
                        nc.vector.tensor_copy(osb[:], acc[:])
                        nc.sync.dma_start(ot[g * SG : (g + 1) * SG], osb[:])
                    else:
                        # chunked drain: chunk j's copy starts as soon as
                        # its own chain's stop-matmul retires
                        rows = ot[g * SG : (g + 1) * SG]
                        for j in range(NB):
                            c0, c1 = j * BN, (j + 1) * BN
                            osbc = small.tile([SG, BN], f32, tag="osbc", bufs=4,
                                              name=f"osbc{j}")
                            if j == NB - 1:
                                # critical path: idle DVE + the uncontended
                                # scalar HWDGE queue for the final output
                                nc.vector.tensor_copy(osbc[:], accb[j][:])
                                nc.scalar.dma_start(rows[:, c0:c1], osbc[:])
                            else:
                                nc.scalar.copy(osbc[:], accb[j][:])
                                nc.sync.dma_start(rows[:, c0:c1], osbc[:])

    nc.compile()
    return nc


def _device_channel_sums(raw, rect, trace=False):
    """Run the bass kernel on 8 cores; return (sums_raw, sums_rect) [128, 2048]
    and the BassKernelResults."""
    from concourse.bass_utils import run_bass_kernel_spmd

    if "nc" not in _CACHE:
        _CACHE["nc"] = _build_nc()
    nc = _CACHE["nc"]

    raw5 = raw.reshape(N_CORES, NS, CB, P, F)
    rect5 = rect.reshape(N_CORES, NS, CB, P, F)
    in_maps = [{"x_raw": raw5[i], "x_rect": rect5[i]} for i in range(N_CORES)]
    res = run_bass_kernel_spmd(nc, in_maps, list(range(N_CORES)), trace=trace)

    sums_raw = np.concatenate([res.results[i]["out_raw"] for i in range(N_CORES)])
    sums_rect = np.concatenate([res.results[i]["out_rect"] for i in range(N_CORES)])
    return sums_raw, sums_rect, res


def _rank_from_sums(sums):
    # channel mean (exact: /256 is a power of two), normalize, svd, count
    m = (sums / np.float32(C)).astype(np.float32)
    nrm = np.linalg.norm(m, axis=1, keepdims=True)
    normed = (m / nrm).reshape(-1, H, W)
    s = np.linalg.svd(normed.astype(np.float32), compute_uv=False)
    return (s > 0.0).sum(axis=1).astype(np.float32)


def kernel(raw_feat, rectified_feat, trace=False):
    raw = np.ascontiguousarray(np.asarray(raw_feat, dtype=np.float32))
    rect = np.ascontiguousarray(np.asarray(rectified_feat, dtype=np.float32))

    sums_raw, sums_rect, res = _device_channel_sums(raw, rect, trace=trace)
    _CACHE["last_results"] = res
    _CACHE["last_sums"] = (sums_raw, sums_rect)

    rank1 = _rank_from_sums(sums_raw)
    rank2 = _rank_from_sums(sums_rect)
    loss = np.maximum(np.float32(0.0), -(rank1 - rank2))
    loss = loss.sum(dtype=np.float32) / np.float32(raw.shape[0])
    return np.asarray(loss, dtype=np.float32)


# revision 10
# speedup vs baseline: 1.1040x; 1.0412x over previous
"""Bass/Trainium2 kernel for nn_LowRankLoss.

Reference computation:
  m      = mean(feat, axis=1)                      # [n, h, w], channel mean
  normed = m / ||m||_F (per sample)
  rank   = #(singular values of normed > 0)        # [n]
  loss   = sum(max(0, -(rank1 - rank2))) / n

The memory-bound part (target_regime=memory) is the channel-mean reduction
over two [128, 256, 32, 64] f32 tensors (512 MiB total). That runs on 8
NeuronCores, data-parallel over the batch dim (16 samples/core). The device
returns per-sample channel sums [n, 2048]; the tiny per-sample SVDs
(128 matrices of 32x64) and the scalar loss are finished on host.

Device design per core (per input tensor, viewed [NS=16, 2, 128, F=2048]):
  - Two contiguous 1 MiB DMAs per sample -> SBUF [128, 2048] x2. 8 KiB
    per-partition descriptors: measured optimal (16 KiB descriptors
    trigger a ~20% slowdown on SDMA engine 15 - the SWDGE descriptor-ring
    port pathology - and the statically balanced work then drags the
    whole stream). SWDGE (gpsimd) issues the steady-state input DMAs
    across all 16 SDMA engines; sample 0 rides the HWDGE rings
    (sync/scalar, shorter first-byte) while the Q7 SWDGE path spins up,
    and is issued BEFORE the one-hot stationary load so input bytes flow
    at the earliest point.
  - TensorE reduces all 256 channels per sample directly from the raw
    input tiles: the two channel-half tiles are bitcast f32->f32r (same
    bits; the PE truncates the mantissa while streaming 1 cycle/row vs 4
    for fp32) and each contributes 4 bank-chunk matmuls accumulated in
    PSUM. Stationary S_m [128, 8] is all-ones in column m = s%8 and zero
    elsewhere, so sample s lands in PSUM row m while other rows
    accumulate +0. No VectorE fold at all: 16-matmul accumulation chains
    (8 samples x 2 halves) per 512-col PSUM bank.
  - acc -> SBUF copy -> DMA out per group.
  - Tail: everything after the last input byte is serial latency. The
    last group uses per-bank PSUM tiles; after the final 1 MiB tile
    lands, only 4 chunk matmuls + per-bank copies + chunked output DMAs
    remain. Chunks 0-2 copy on ACT (scalar) so the final chunk's DVE
    copy is never queued, and the final output chunk goes out on its own
    HWDGE queue. The remaining ~8.5 us after the last output DMA is the
    fixed framework epilogue (DMA-lane drains, all-engine barriers, a
    constant ~4.2 us quiesce wait, semaphore-file resets) and does not
    depend on the kernel body.
f32r truncates the data mantissa (~1e-4 rel err on the channel sums),
far below what could flip a singular-value-positivity count (min sigma
~2e-2 here).
"""

import numpy as np

N_CORES = 8
NS = 16           # samples per core
C = 256           # channels
H, W = 32, 64
F = H * W         # 2048 spatial
CB = 2            # channel halves
P = 128           # partitions
SG = 8            # samples per PSUM group
NB = 4            # matmuls per half-sample (N=512 PSUM bank limit)
BN = F // NB      # 512

_CACHE = {}


def _build_nc():
    import concourse.bacc as bacc
    import concourse.mybir as mybir
    import concourse.tile as tile

    nc = bacc.Bacc(None, target_bir_lowering=False)
    f32 = mybir.dt.float32
    f32r = mybir.dt.float32r

    x_raw = nc.dram_tensor("x_raw", [NS, CB, P, F], f32, kind="ExternalInput")
    x_rect = nc.dram_tensor("x_rect", [NS, CB, P, F], f32, kind="ExternalInput")
    out_raw = nc.dram_tensor("out_raw", [NS, F], f32, kind="ExternalOutput")
    out_rect = nc.dram_tensor("out_rect", [NS, F], f32, kind="ExternalOutput")

    with tile.TileContext(nc) as tc:
        with (
            tc.tile_pool(name="io", bufs=9) as pool,
            tc.tile_pool(name="small", bufs=2) as small,
            tc.tile_pool(name="psum", bufs=1, space="PSUM") as psum,
        ):
            # Sample 0 rides the HWDGE rings before anything else so input
            # bytes start flowing at the earliest possible point.
            t0_first = pool.tile([P, F], f32r, tag="in0")
            t1_first = pool.tile([P, F], f32r, tag="in1")
            nc.sync.dma_start(t0_first[:], x_raw[0, 0].bitcast(f32r))
            nc.scalar.dma_start(t1_first[:], x_raw[0, 1].bitcast(f32r))

            # C[k, 8m + j] = 1 if j == m else 0; lhsT for sample s is the
            # [128, 8] slice C[:, 8m:8m+8] with m = s % 8.
            s_np = np.zeros((P, SG * SG), np.float32)
            for m in range(SG):
                s_np[:, SG * m + m] = 1.0
            s_dram = nc.inline_tensor(s_np, name="s_const")
            s_stage = small.tile([P, SG * SG], f32, tag="stat_stage")
            nc.sync.dma_start(s_stage[:], s_dram[:])
            S = small.tile([P, SG * SG], f32r, tag="stat")
            nc.vector.tensor_copy(S[:], s_stage[:])

            for xt, ot in ((x_raw, out_raw), (x_rect, out_rect)):
                for g in range(NS // SG):
                    last_group = xt is x_rect and g == NS // SG - 1
                    if not last_group:
                        acc = psum.tile([SG, F], f32, tag="acc", bufs=1)
                        accb = [acc[:, j * BN : (j + 1) * BN] for j in range(NB)]
                    else:
                        # per-bank PSUM tiles so each 512-col chunk's copy
                        # only waits on its own accumulation chain
                        accb = [
                            psum.tile([SG, BN], f32, tag=f"accc{j}", bufs=1,
                                      name=f"accc{j}")
                            for j in range(NB)
                        ]
                    for m in range(SG):
                        s = g * SG + m
                        if s == 0 and xt is x_raw:
                            t0, t1 = t0_first, t1_first
                        else:
                            t0 = pool.tile([P, F], f32r, tag="in0")
                            t1 = pool.tile([P, F], f32r, tag="in1")
                            nc.gpsimd.dma_start(t0[:], xt[s, 0].bitcast(f32r))
                            nc.gpsimd.dma_start(t1[:], xt[s, 1].bitcast(f32r))
                        for h, t in ((0, t0), (1, t1)):
                            tm = t
                            for j in range(NB):
                                nc.tensor.matmul(
                                    accb[j],
                                    S[:, SG * m : SG * m + SG],
                                    tm[:, j * BN : (j + 1) * BN],
                                    start=(m == 0 and h == 0),
                                    stop=(m == SG - 1 and h == 1),
                                )
                    if not last_group:
                        osb = small.tile([SG, F], f32, tag="osb")
                        nc.vector.tensor_copy(osb[:], acc[:])
                        # SWDGE spreads the 64 KiB across all 16 engines
                        # (HWDGE outputs pile onto engines 0-7); the queue
                        # wait is absorbed by the ~20 us descriptor backlog
                        nc.gpsimd.dma_start(ot[g * SG : (g + 1) * SG], osb[:])
                    else:
                        # chunked drain: chunk j's copy starts as soon as its
                        # own chain's stop-matmul retires, alternating ACT/DVE
                        # so two copies run concurrently; subtile deps let
                        # each half-width output DMA leave once its two
                        # copies land. All outs on sync (fastest DIRECT2D).
                        rows = ot[g * SG : (g + 1) * SG]
                        osb = small.tile([SG, F], f32, tag="osbL")
                        for j in range(NB):
                            c0, c1 = j * BN, (j + 1) * BN
                            if j % 2 == 0:
                                nc.scalar.copy(osb[:, c0:c1], accb[j][:])
                            else:
                                nc.vector.tensor_copy(osb[:, c0:c1], accb[j][:])
                        half = F // 2
                        nc.sync.dma_start(rows[:, 0:half], osb[:, 0:half])
                        nc.sync.dma_start(rows[:, half:F], osb[:, half:F])

    nc.compile()
    return nc


def _device_channel_sums(raw, rect, trace=False):
    """Run the bass kernel on 8 cores; return (sums_raw, sums_rect) [128, 2048]
    and the BassKernelResults."""
    from concourse.bass_utils import run_bass_kernel_spmd

    if "nc" not in _CACHE:
        _CACHE["nc"] = _build_nc()
    nc = _CACHE["nc"]

    raw5 = raw.reshape(N_CORES, NS, CB, P, F)
    rect5 = rect.reshape(N_CORES, NS, CB, P, F)
    in_maps = [{"x_raw": raw5[i], "x_rect": rect5[i]} for i in range(N_CORES)]
    res = run_bass_kernel_spmd(nc, in_maps, list(range(N_CORES)), trace=trace)

    sums_raw = np.concatenate([res.results[i]["out_raw"] for i in range(N_CORES)])
    sums_rect = np.concatenate([res.results[i]["out_rect"] for i in range(N_CORES)])
    return sums_raw, sums_rect, res


def _rank_from_sums(sums):
    # channel mean (exact: /256 is a power of two), normalize, svd, count
    m = (sums / np.float32(C)).astype(np.float32)
    nrm = np.linalg.norm(m, axis=1, keepdims=True)
    normed = (m / nrm).reshape(-1, H, W)
    s = np.linalg.svd(normed.astype(np.float32), compute_uv=False)
    return (s > 0.0).sum(axis=1).astype(np.float32)


def kernel(raw_feat, rectified_feat, trace=False):
    raw = np.ascontiguousarray(np.asarray(raw_feat, dtype=np.float32))
    rect = np.ascontiguousarray(np.asarray(rectified_feat, dtype=np.float32))

    sums_raw, sums_rect, res = _device_channel_sums(raw, rect, trace=trace)
    _CACHE["last_results"] = res
    _CACHE["last_sums"] = (sums_raw, sums_rect)

    rank1 = _rank_from_sums(sums_raw)
    rank2 = _rank_from_sums(sums_rect)
    loss = np.maximum(np.float32(0.0), -(rank1 - rank2))
    loss = loss.sum(dtype=np.float32) / np.float32(raw.shape[0])
    return np.asarray(loss, dtype=np.float32)


# revision 11
# speedup vs baseline: 1.1762x; 1.0653x over previous
"""Bass/Trainium2 kernel for nn_LowRankLoss.

Reference computation:
  m      = mean(feat, axis=1)                      # [n, h, w], channel mean
  normed = m / ||m||_F (per sample)
  rank   = #(singular values of normed > 0)        # [n]
  loss   = sum(max(0, -(rank1 - rank2))) / n

The memory-bound part (target_regime=memory) is the channel-mean reduction
over two [128, 256, 32, 64] f32 tensors (512 MiB total). That runs on 8
NeuronCores, data-parallel over the batch dim (16 samples/core). The device
returns per-sample channel sums [n, 2048]; the tiny per-sample SVDs
(128 matrices of 32x64) and the scalar loss are finished on host.

Device design per core (per input tensor, viewed [NS=16, 2, 128, F=2048]):
  - Two contiguous 1 MiB DMAs per sample -> SBUF [128, 2048] x2. 8 KiB
    per-partition descriptors: measured optimal (16 KiB descriptors
    trigger a ~20% slowdown on SDMA engine 15 - the SWDGE descriptor-ring
    port pathology - and the statically balanced work then drags the
    whole stream). SWDGE (gpsimd) issues the steady-state input DMAs
    across all 16 SDMA engines; sample 0 rides the HWDGE rings
    (sync/scalar, shorter first-byte) while the Q7 SWDGE path spins up,
    and is issued BEFORE the one-hot stationary load so input bytes flow
    at the earliest point.
  - TensorE reduces all 256 channels per sample directly from the raw
    input tiles: the two channel-half tiles are bitcast f32->f32r (same
    bits; the PE truncates the mantissa while streaming 1 cycle/row vs 4
    for fp32) and each contributes 4 bank-chunk matmuls accumulated in
    PSUM. Stationary S_m [128, 8] is all-ones in column m = s%8 and zero
    elsewhere, so sample s lands in PSUM row m while other rows
    accumulate +0. No VectorE fold at all: 16-matmul accumulation chains
    (8 samples x 2 halves) per 512-col PSUM bank.
  - acc -> SBUF copy -> DMA out per group.
  - Tail: everything after the last input byte is serial latency. The
    last group uses per-bank PSUM tiles; after the final 1 MiB tile
    lands, only 4 chunk matmuls + per-bank copies + chunked output DMAs
    remain. Chunks 0-2 copy on ACT (scalar) so the final chunk's DVE
    copy is never queued, and the final output chunk goes out on its own
    HWDGE queue. The remaining ~8.5 us after the last output DMA is the
    fixed framework epilogue (DMA-lane drains, all-engine barriers, a
    constant ~4.2 us quiesce wait, semaphore-file resets) and does not
    depend on the kernel body.
f32r truncates the data mantissa (~1e-4 rel err on the channel sums),
far below what could flip a singular-value-positivity count (min sigma
~2e-2 here).
"""

import numpy as np

N_CORES = 8
NS = 16           # samples per core
C = 256           # channels
H, W = 32, 64
F = H * W         # 2048 spatial
CB = 2            # channel halves
P = 128           # partitions
SG = 8            # samples per PSUM group
NB = 4            # matmuls per half-sample (N=512 PSUM bank limit)
BN = F // NB      # 512

_CACHE = {}


def _build_nc():
    import concourse.bacc as bacc
    import concourse.mybir as mybir
    import concourse.tile as tile

    nc = bacc.Bacc(None, target_bir_lowering=False)
    f32 = mybir.dt.float32
    f32r = mybir.dt.float32r

    x_raw = nc.dram_tensor("x_raw", [NS, CB, P, F], f32, kind="ExternalInput")
    x_rect = nc.dram_tensor("x_rect", [NS, CB, P, F], f32, kind="ExternalInput")
    out_raw = nc.dram_tensor("out_raw", [NS, F], f32, kind="ExternalOutput")
    out_rect = nc.dram_tensor("out_rect", [NS, F], f32, kind="ExternalOutput")

    with tile.TileContext(nc) as tc:
        with (
            tc.tile_pool(name="io", bufs=9) as pool,
            tc.tile_pool(name="small", bufs=2) as small,
            tc.tile_pool(name="psum", bufs=1, space="PSUM") as psum,
        ):
            # C[k, 8m + j] = 1 if j == m else 0; lhsT for sample s is the
            # [128, 8] slice C[:, 8m:8m+8] with m = s % 8.
            s_np = np.zeros((P, SG * SG), np.float32)
            for m in range(SG):
                s_np[:, SG * m + m] = 1.0
            s_dram = nc.inline_tensor(s_np, name="s_const")
            s_stage = small.tile([P, SG * SG], f32, tag="stat_stage")
            nc.sync.dma_start(s_stage[:], s_dram[:])
            S = small.tile([P, SG * SG], f32r, tag="stat")
            nc.vector.tensor_copy(S[:], s_stage[:])

            for xt, ot in ((x_raw, out_raw), (x_rect, out_rect)):
                for g in range(NS // SG):
                    last_group = xt is x_rect and g == NS // SG - 1
                    if not last_group:
                        acc = psum.tile([SG, F], f32, tag="acc", bufs=1)
                        accb = [acc[:, j * BN : (j + 1) * BN] for j in range(NB)]
                    else:
                        # per-bank PSUM tiles so each 512-col chunk's copy
                        # only waits on its own accumulation chain
                        accb = [
                            psum.tile([SG, BN], f32, tag=f"accc{j}", bufs=1,
                                      name=f"accc{j}")
                            for j in range(NB)
                        ]
                    for m in range(SG):
                        s = g * SG + m
                        # all inputs on the one SWDGE queue: every 1 MiB DMA
                        # spreads 16 descriptors to each of the 16 engines,
                        # keeping the per-engine load exactly balanced (a
                        # HWDGE head start lands on engines 0-7 only and
                        # delays the shared stream's completion)
                        t0 = pool.tile([P, F], f32r, tag="in0")
                        t1 = pool.tile([P, F], f32r, tag="in1")
                        nc.gpsimd.dma_start(t0[:], xt[s, 0].bitcast(f32r))
                        nc.gpsimd.dma_start(t1[:], xt[s, 1].bitcast(f32r))
                        for h, t in ((0, t0), (1, t1)):
                            tm = t
                            for j in range(NB):
                                nc.tensor.matmul(
                                    accb[j],
                                    S[:, SG * m : SG * m + SG],
                                    tm[:, j * BN : (j + 1) * BN],
                                    start=(m == 0 and h == 0),
                                    stop=(m == SG - 1 and h == 1),
                                )
                    if not last_group:
                        osb = small.tile([SG, F], f32, tag="osb")
                        nc.vector.tensor_copy(osb[:], acc[:])
                        # SWDGE spreads the 64 KiB across all 16 engines
                        # (HWDGE outputs pile onto engines 0-7); the queue
                        # wait is absorbed by the ~20 us descriptor backlog
                        nc.gpsimd.dma_start(ot[g * SG : (g + 1) * SG], osb[:])
                    else:
                        # chunked drain: chunk j's copy starts as soon as its
                        # own chain's stop-matmul retires, alternating ACT/DVE
                        # so two copies run concurrently; subtile deps let
                        # each half-width output DMA leave once its two
                        # copies land. All outs on sync (fastest DIRECT2D).
                        rows = ot[g * SG : (g + 1) * SG]
                        osb = small.tile([SG, F], f32, tag="osbL")
                        for j in range(NB):
                            c0, c1 = j * BN, (j + 1) * BN
                            if j % 2 == 0:
                                nc.scalar.copy(osb[:, c0:c1], accb[j][:])
                            else:
                                nc.vector.tensor_copy(osb[:, c0:c1], accb[j][:])
                        half = F // 2
                        nc.sync.dma_start(rows[:, 0:half], osb[:, 0:half])
                        nc.sync.dma_start(rows[:, half:F], osb[:, half:F])

    nc.compile()
    return nc


def _device_channel_sums(raw, rect, trace=False):
    """Run the bass kernel on 8 cores; return (sums_raw, sums_rect) [128, 2048]
    and the BassKernelResults."""
    from concourse.bass_utils import run_bass_kernel_spmd

    if "nc" not in _CACHE:
        _CACHE["nc"] = _build_nc()
    nc = _CACHE["nc"]

    raw5 = raw.reshape(N_CORES, NS, CB, P, F)
    rect5 = rect.reshape(N_CORES, NS, CB, P, F)
    in_maps = [{"x_raw": raw5[i], "x_rect": rect5[i]} for i in range(N_CORES)]
    res = run_bass_kernel_spmd(nc, in_maps, list(range(N_CORES)), trace=trace)

    sums_raw = np.concatenate([res.results[i]["out_raw"] for i in range(N_CORES)])
    sums_rect = np.concatenate([res.results[i]["out_rect"] for i in range(N_CORES)])
    return sums_raw, sums_rect, res


def _rank_from_sums(sums):
    # channel mean (exact: /256 is a power of two), normalize, svd, count
    m = (sums / np.float32(C)).astype(np.float32)
    nrm = np.linalg.norm(m, axis=1, keepdims=True)
    normed = (m / nrm).reshape(-1, H, W)
    s = np.linalg.svd(normed.astype(np.float32), compute_uv=False)
    return (s > 0.0).sum(axis=1).astype(np.float32)


def kernel(raw_feat, rectified_feat, trace=False):
    raw = np.ascontiguousarray(np.asarray(raw_feat, dtype=np.float32))
    rect = np.ascontiguousarray(np.asarray(rectified_feat, dtype=np.float32))

    sums_raw, sums_rect, res = _device_channel_sums(raw, rect, trace=trace)
    _CACHE["last_results"] = res
    _CACHE["last_sums"] = (sums_raw, sums_rect)

    rank1 = _rank_from_sums(sums_raw)
    rank2 = _rank_from_sums(sums_rect)
    loss = np.maximum(np.float32(0.0), -(rank1 - rank2))
    loss = loss.sum(dtype=np.float32) / np.float32(raw.shape[0])
    return np.asarray(loss, dtype=np.float32)
